# revision 3
# baseline (speedup 1.0000x reference)
"""Causal GQA self-attention (B=2, S=2048, H=2048, 16 q-heads / 4 kv-heads,
head_dim=128, RoPE) as a Bass/Tile kernel on 8 TRN2 NeuronCores.

v2 sharding: hybrid batch x head-group tensor parallel. Core c owns batch
b = c//4 and head-group g = c%4 (q-heads 4g..4g+3, kv-head g -- exactly the
GQA grouping, so no kv replication). Each core computes a full [S, H]
partial of its batch's output projection; the host sums 4 partials per
batch.

On-chip layout mirrors v1 (q/k produced transposed from the PE, RoPE
rotate-half as a signed-permutation matmul, v back to natural layout via PE
transposes, scores transposed, max-free softmax) with these changes:
 - all inputs bf16 and pre-tiled on the host so every DMA is contiguous.
 - softmax denominator via DVE accumulation of the exp tiles plus a single
   ones-matmul on the accumulated sum (instead of a full ones-matmul per
   k-block): removes ~1/3 of the attention-phase PE work.
 - reciprocal_approx_fast for 1/den (~5x faster than reciprocal; den is a
   sum of exp() values in [1, ~1e4], far from the undefined edge cases).
 - output projection of the first row-half is interleaved with the second
   half's attention units to backfill the PE while ACT (exp) is busy.
 - output stored bf16, host sums partials in f32.
"""

import math

import numpy as np
import ml_dtypes

import concourse.bass as bass
import concourse.tile as tile
from concourse import mybir
from concourse.bass_utils import run_bass_kernel_spmd

F32 = mybir.dt.float32
F32R = mybir.dt.float32r
BF16 = mybir.dt.bfloat16
AF = mybir.ActivationFunctionType

B, S, H = 2, 2048, 2048
NH, NKV, HD = 16, 4, 128
N_CORES = 8
NHL = 4                # q heads per core
KT = H // 128          # 16 k-tiles over the H contraction
SC = 512               # proj s-chunk width
NSC = S // SC          # 4
QT = 1024              # attention qi tile width
NQT = S // QT          # 2
SCALE = 1.0 / math.sqrt(HD)
ROPE_BASE = 10000.0


def _alu(name):
    from concourse.alu_op_type import AluOpType

    return getattr(AluOpType, name)


def legalize_waits(nc, cap=1):
    """walrus in this container accepts at most one sync-wait per
    instruction; move excess waits onto NoOp carriers just before the
    instruction on the same engine (sequencers run waits in order, so this
    is semantically identical)."""
    n_split = 0
    for f in nc.m.functions:
        for blk in f.blocks:
            if not any(
                i.sync_info is not None and len(i.sync_info.on_wait) > cap
                for i in blk.instructions
            ):
                continue
            new_insts = []
            for inst in blk.instructions:
                si = inst.sync_info
                waits = list(si.on_wait) if si is not None else []
                if len(waits) > cap:
                    for k, w in enumerate(waits[:-cap]):
                        new_insts.append(
                            mybir.InstNoOp(
                                name=f"{inst.name}-wsplit{k}",
                                engine=inst.engine,
                                sync_info=mybir.SyncInfo(on_wait=[w], on_update=[]),
                            )
                        )
                        n_split += 1
                    inst.sync_info = mybir.SyncInfo(
                        on_wait=waits[-cap:], on_update=list(si.on_update)
                    )
                new_insts.append(inst)
            blk.instructions = new_insts
    return n_split


def build_nc(legalize=True):
    mult = _alu("mult")
    add = _alu("add")

    nc = bass.Bass(trn_type="TRN2", target_bir_lowering=False)

    x4_d = nc.dram_tensor("x4", [NSC, 128, KT, SC], BF16, kind="ExternalInput")
    wq_d = nc.dram_tensor("wq", [128, KT, NHL * HD], BF16, kind="ExternalInput")
    wk_d = nc.dram_tensor("wk", [128, KT, HD], BF16, kind="ExternalInput")
    wv_d = nc.dram_tensor("wv", [128, KT, HD], BF16, kind="ExternalInput")
    wo_d = nc.dram_tensor("wo", [128, NHL, H], BF16, kind="ExternalInput")
    cos_d = nc.dram_tensor("cosT", [HD, S], BF16, kind="ExternalInput")
    sinrot_d = nc.dram_tensor("sinrotT", [HD, S], F32, kind="ExternalInput")
    mask_d = nc.dram_tensor("addmask", [128, 128], F32, kind="ExternalInput")
    rotm_d = nc.dram_tensor("rotmT", [128, 128], BF16, kind="ExternalInput")
    iden_d = nc.dram_tensor("iden", [128, 128], BF16, kind="ExternalInput")
    magic_d = nc.dram_tensor("magic", [128, SC], mybir.dt.int32, kind="ExternalInput")
    ones_d = nc.dram_tensor("ones", [128, 128], F32R, kind="ExternalInput")
    o_d = nc.dram_tensor("o", [S // 128, 128, 4, 512], BF16, kind="ExternalOutput")

    with tile.TileContext(nc) as tc:
        with (
            tc.tile_pool(name="consts", bufs=1) as consts,
            tc.tile_pool(name="xpool", bufs=2) as xpool,
            tc.tile_pool(name="homes", bufs=1) as homes,
            tc.tile_pool(name="stage", bufs=4) as stage,
            tc.tile_pool(name="tmpp", bufs=3) as tmpp,
            tc.tile_pool(name="accp", bufs=2) as accp,
            tc.tile_pool(name="denrp", bufs=2) as denrp,
            tc.tile_pool(name="ptp", bufs=3) as ptp,
            tc.tile_pool(name="opool", bufs=3) as opool,
            tc.tile_pool(name="ps", bufs=1, space="PSUM") as ps,
        ):
            # ---- constants (emission order = rough DMA priority) ----
            wq_sb = consts.tile([128, KT, NHL * HD], BF16, tag="wq")
            nc.sync.dma_start(out=wq_sb, in_=wq_d.ap())
            wk_sb = consts.tile([128, KT, HD], BF16, tag="wk")
            nc.sync.dma_start(out=wk_sb, in_=wk_d.ap())
            wv_sb = consts.tile([128, KT, HD], BF16, tag="wv")
            nc.sync.dma_start(out=wv_sb, in_=wv_d.ap())
            rotm_sb = consts.tile([128, 128], BF16, tag="rotm")
            nc.sync.dma_start(out=rotm_sb, in_=rotm_d.ap())
            iden_sb = consts.tile([128, 128], BF16, tag="iden")
            nc.sync.dma_start(out=iden_sb, in_=iden_d.ap())
            mask_sb = consts.tile([128, 128], F32, tag="mask")
            nc.sync.dma_start(out=mask_sb, in_=mask_d.ap())
            cos_sb = consts.tile([HD, S], BF16, tag="cos")
            nc.sync.dma_start(out=cos_sb, in_=cos_d.ap())
            sinrot_sb = consts.tile([HD, S], F32, tag="sinrot")
            nc.sync.dma_start(out=sinrot_sb, in_=sinrot_d.ap())
            wo_sb = consts.tile([128, NHL, H], BF16, tag="wo")
            nc.sync.dma_start(out=wo_sb, in_=wo_d.ap())
            ones_sb = consts.tile([128, 128], F32R, tag="ones")
            nc.sync.dma_start(out=ones_sb, in_=ones_d.ap())
            # int32 magic-constant tile for the reciprocal seed
            magic_sb = consts.tile([128, SC], mybir.dt.int32, tag="magic")
            nc.sync.dma_start(out=magic_sb, in_=magic_d.ap())

            # ---- homes ----
            q_homes = [
                homes.tile([HD, S], BF16, tag=f"q{h}", name=f"q{h}_sb")
                for h in range(NHL)
            ]
            kT_sb = homes.tile([HD, S], BF16, tag="kT")
            vp_sb = homes.tile([128, KT, HD], BF16, tag="vp")
            aT_sb = homes.tile([128, NHL, S], BF16, tag="aT")

            # ================= phase A: QKV projections + RoPE ========
            for c in range(NSC):
                cs = slice(c * SC, (c + 1) * SC)
                xc = xpool.tile([128, KT, SC], BF16, tag="xc")
                nc.sync.dma_start(out=xc, in_=x4_d.ap()[c])

                # k/v projections
                pskv = ps.tile([128, 2 * SC], F32, tag="sT")
                for k in range(KT):
                    st = dict(start=(k == 0), stop=(k == KT - 1))
                    nc.tensor.matmul(pskv[:, 0:SC], wk_sb[:, k, :], xc[:, k, :], **st)
                    nc.tensor.matmul(pskv[:, SC:2 * SC], wv_sb[:, k, :], xc[:, k, :], **st)
                kraw = stage.tile([128, SC], BF16, tag="raw")
                nc.scalar.copy(kraw, pskv[:, 0:SC])
                vt = stage.tile([128, SC], BF16, tag="raw")
                nc.scalar.copy(vt, pskv[:, SC:2 * SC])
                # v: transpose to natural layout via PE (4x 128x128)
                pvt = ps.tile([128, 2 * SC], BF16, tag="small")
                for j2 in range(SC // 128):
                    nc.tensor.transpose(
                        pvt[:, j2 * 128:(j2 + 1) * 128],
                        vt[:, j2 * 128:(j2 + 1) * 128],
                        iden_sb,
                    )
                nc.vector.tensor_copy(
                    vp_sb[:, c * (SC // 128):(c + 1) * (SC // 128), :],
                    pvt[:, 0:SC],
                )
                # k rope
                psrk = ps.tile([128, SC], F32, tag="small")
                nc.tensor.matmul(psrk, rotm_sb, kraw, start=True, stop=True)
                ktmp = tmpp.tile([128, SC], BF16, tag="tmp")
                nc.vector.tensor_tensor(ktmp, psrk, sinrot_sb[:, cs], mult)
                nc.vector.tensor_tensor(kT_sb[:, cs], kraw, cos_sb[:, cs], mult)
                nc.vector.tensor_tensor(kT_sb[:, cs], kT_sb[:, cs], ktmp, add)

                # q projections + rope, head pairs to fit PSUM
                for pr in range(2):
                    h0, h1 = 2 * pr, 2 * pr + 1
                    psq = ps.tile([128, 2 * SC], F32, tag="sT")
                    for k in range(KT):
                        st = dict(start=(k == 0), stop=(k == KT - 1))
                        nc.tensor.matmul(
                            psq[:, 0:SC], wq_sb[:, k, h0 * HD:(h0 + 1) * HD],
                            xc[:, k, :], **st,
                        )
                        nc.tensor.matmul(
                            psq[:, SC:2 * SC], wq_sb[:, k, h1 * HD:(h1 + 1) * HD],
                            xc[:, k, :], **st,
                        )
                    qraw0 = stage.tile([128, SC], BF16, tag="raw")
                    nc.scalar.copy(qraw0, psq[:, 0:SC])
                    qraw1 = stage.tile([128, SC], BF16, tag="raw")
                    nc.scalar.copy(qraw1, psq[:, SC:2 * SC])
                    psrot = ps.tile([128, 2 * SC], F32, tag="sT")
                    nc.tensor.matmul(psrot[:, 0:SC], rotm_sb, qraw0, start=True, stop=True)
                    nc.tensor.matmul(psrot[:, SC:2 * SC], rotm_sb, qraw1, start=True, stop=True)
                    for i, (qraw, h) in enumerate(((qraw0, h0), (qraw1, h1))):
                        qtmp = tmpp.tile([128, SC], BF16, tag="tmp")
                        nc.vector.tensor_tensor(
                            qtmp, psrot[:, i * SC:(i + 1) * SC], sinrot_sb[:, cs], mult
                        )
                        nc.vector.tensor_tensor(q_homes[h][:, cs], qraw, cos_sb[:, cs], mult)
                        nc.vector.tensor_tensor(
                            q_homes[h][:, cs], q_homes[h][:, cs], qtmp, add
                        )

            # ================= phase B/C helpers ====================
            def attn_unit(h, t):
                qi0 = t * QT
                nblk = (qi0 + QT) // 128
                qh = q_homes[h]
                seg_touchers = {}
                for s0 in range(0, QT, SC):
                    js = [
                        j for j in range(nblk)
                        if max(j * 128 - qi0, 0) < s0 + SC
                    ]
                    seg_touchers[s0] = (js[0], js[-1])
                acc = accp.tile([128, QT], F32R, tag="acc")
                outT = ps.tile([128, QT], F32, tag="outT")
                for j in range(nblk):
                    kj0 = j * 128
                    r = kj0 - qi0
                    c0 = max(r, 0)
                    sT = ps.tile([128, QT], F32, tag="sT")
                    for s0 in range(0, QT, SC):
                        a0, a1 = max(c0, s0), s0 + SC
                        if a0 >= a1:
                            continue
                        nc.tensor.matmul(
                            sT[:, a0:a1],
                            kT_sb[:, kj0:kj0 + 128],
                            qh[:, qi0 + a0:qi0 + a1],
                            start=True, stop=True,
                        )
                    if r >= 0:
                        nc.vector.tensor_tensor(
                            sT[:, c0:c0 + 128], sT[:, c0:c0 + 128], mask_sb, add
                        )
                    pt = ptp.tile([128, QT], BF16, tag="pt")
                    nc.scalar.activation(
                        out=pt[:, c0:QT], in_=sT[:, c0:QT], func=AF.Exp, scale=SCALE
                    )
                    if j == 0:
                        nc.vector.tensor_copy(acc, pt)
                    else:
                        nc.vector.tensor_tensor(
                            acc[:, c0:QT], acc[:, c0:QT], pt[:, c0:QT], add
                        )
                    for s0 in range(0, QT, SC):
                        a0, a1 = max(c0, s0), s0 + SC
                        if a0 >= a1:
                            continue
                        jf, jl = seg_touchers[s0]
                        nc.tensor.matmul(
                            outT[:, a0:a1], vp_sb[:, j, :], pt[:, a0:a1],
                            start=(j == jf), stop=(j == jl),
                        )
                # normalize: den = colsum(acc) via ones-matmul, then 1/den by
                # magic-constant seed (0x7EF127EA - bits, ~5% err) + one
                # Newton-Raphson pass (~0.3% worst case), standard DVE ops.
                sub = _alu("subtract")
                for half in range(2):
                    e0 = half * SC
                    den = ps.tile([128, SC], F32, tag="small")
                    nc.tensor.matmul(
                        den, ones_sb, acc[:, e0:e0 + SC], start=True, stop=True
                    )
                    denr = denrp.tile([128, SC], F32, tag="denr")
                    # seed: bits(y0) = MAGIC - bits(den)
                    nc.vector.tensor_tensor(
                        denr.bitcast(mybir.dt.int32),
                        magic_sb,
                        den.bitcast(mybir.dt.int32),
                        sub,
                    )
                    # NR: y1 = y0 * (2 - d*y0)
                    dnt = denrp.tile([128, SC], F32, tag="dnt")
                    nc.vector.scalar_tensor_tensor(
                        dnt, den, -1.0, denr, mult, mult,
                    )
                    nc.vector.scalar_tensor_tensor(
                        denr, dnt, 2.0, denr, add, mult,
                    )
                    nc.vector.tensor_tensor(
                        aT_sb[:, h, qi0 + e0:qi0 + e0 + SC],
                        outT[:, e0:e0 + SC], denr, mult,
                    )

            def oproj_tile(mt):
                ms = slice(mt * 128, (mt + 1) * 128)
                for n0 in range(4):
                    pso = ps.tile([128, 512], F32, tag="small")
                    for ci in range(NHL):
                        nc.tensor.matmul(
                            pso,
                            aT_sb[:, ci, ms],
                            wo_sb[:, ci, n0 * 512:(n0 + 1) * 512],
                            start=(ci == 0), stop=(ci == NHL - 1),
                        )
                    os_sb = opool.tile([128, 512], BF16, tag="os")
                    if n0 % 2 == 0:
                        nc.vector.tensor_copy(os_sb, pso)
                    else:
                        nc.scalar.copy(os_sb, pso)
                    nc.sync.dma_start(out=o_d.ap()[mt, :, n0, :], in_=os_sb)

            # ================= phase B t0, then B t1 interleaved with C t0
            for h in range(NHL):
                attn_unit(h, 0)
            for h in range(NHL):
                attn_unit(h, 1)
                oproj_tile(2 * h)
                oproj_tile(2 * h + 1)
            for mt in range(8, 16):
                oproj_tile(mt)

    if legalize:
        legalize_waits(nc)
    return nc


_NC_CACHE = None


def _get_nc():
    global _NC_CACHE
    if _NC_CACHE is None:
        _NC_CACHE = build_nc()
    return _NC_CACHE


def _host_consts():
    inv = 1.0 / (ROPE_BASE ** (np.arange(0, HD, 2, dtype=np.float32) / HD))
    t = np.arange(S, dtype=np.float32)
    freqs = np.outer(t, inv)                       # [S, HD/2]
    emb = np.concatenate([freqs, freqs], axis=-1)  # [S, HD]
    cos = np.cos(emb)
    sin = np.sin(emb)
    cosT = np.ascontiguousarray(cos.T).astype(ml_dtypes.bfloat16)     # [HD, S]
    sinrotT = np.ascontiguousarray(sin.T).astype(np.float32)
    jj, ii = np.meshgrid(np.arange(128), np.arange(128), indexing="ij")
    addmask = np.where(jj <= ii, 0.0, -1e9).astype(np.float32)
    # rot(q)[d] = -q[d+64] (d<64), q[d-64] (d>=64); rot = R @ q and the PE
    # computes lhsT.T @ rhs, so pass R.T as the stationary operand.
    R = np.zeros((128, 128), dtype=np.float32)
    for d in range(64):
        R[d, d + 64] = -1.0
        R[d + 64, d] = 1.0
    rotmT = np.ascontiguousarray(R.T).astype(ml_dtypes.bfloat16)
    iden = np.eye(128, dtype=np.float32).astype(ml_dtypes.bfloat16)
    return cosT, sinrotT, addmask, rotmT, iden


def _tile_w(w):
    """[H, D] f32 -> [128, KT, D] bf16 with w_t[p, k, d] = w[p + 128k, d]."""
    return np.ascontiguousarray(
        w.reshape(KT, 128, -1).transpose(1, 0, 2)
    ).astype(ml_dtypes.bfloat16)


def build_in_maps(x, wq, wk, wv, wo):
    cosT, sinrotT, addmask, rotmT, iden = _host_consts()
    # x4[b][c, p, k, s] = x[b, c*SC + s, p + 128k]
    x4 = [
        np.ascontiguousarray(
            x[b].T.reshape(KT, 128, NSC, SC).transpose(2, 1, 0, 3)
        ).astype(ml_dtypes.bfloat16)
        for b in range(B)
    ]
    in_maps = []
    for c in range(N_CORES):
        b, g = divmod(c, NHL)
        wo_slice = wo[g * NHL * HD:(g + 1) * NHL * HD, :]  # [512, H]
        in_maps.append({
            "x4": x4[b],
            "wq": _tile_w(wq[:, g * NHL * HD:(g + 1) * NHL * HD]),
            "wk": _tile_w(wk[:, g * HD:(g + 1) * HD]),
            "wv": _tile_w(wv[:, g * HD:(g + 1) * HD]),
            "wo": np.ascontiguousarray(
                wo_slice.reshape(NHL, 128, H).transpose(1, 0, 2)
            ).astype(ml_dtypes.bfloat16),
            "cosT": cosT,
            "sinrotT": sinrotT,
            "addmask": addmask,
            "rotmT": rotmT,
            "iden": iden,
            "magic": np.full((128, SC), 0x7EF127EA, dtype=np.int32),
            "ones": np.ones((128, 128), dtype=np.float32),
        })
    return in_maps


def kernel(x, wq, wk, wv, wo):
    x = np.asarray(x, dtype=np.float32)
    wq = np.asarray(wq, dtype=np.float32)
    wk = np.asarray(wk, dtype=np.float32)
    wv = np.asarray(wv, dtype=np.float32)
    wo = np.asarray(wo, dtype=np.float32)

    in_maps = build_in_maps(x, wq, wk, wv, wo)
    nc = _get_nc()
    res = run_bass_kernel_spmd(nc, in_maps, core_ids=list(range(N_CORES)))
    globals()["_LAST_RESULT"] = res
    out = np.zeros((B, S, H), dtype=np.float32)
    for c, r in enumerate(res.results):
        b = c // NHL
        out[b] += r["o"].astype(np.float32).reshape(S, H)
    return out


if __name__ == "__main__":
    rng = np.random.default_rng(0)
    ins = {
        "x": rng.standard_normal((B, S, H), dtype=np.float32),
        "wq": rng.standard_normal((H, NH * HD), dtype=np.float32) * 0.02,
        "wk": rng.standard_normal((H, NKV * HD), dtype=np.float32) * 0.02,
        "wv": rng.standard_normal((H, NKV * HD), dtype=np.float32) * 0.02,
        "wo": rng.standard_normal((NH * HD, H), dtype=np.float32) * 0.02,
    }
    out = kernel(**ins)
    print("out", out.shape, out.dtype, float(np.abs(out).max()))


# revision 4
# speedup vs baseline: 1.6430x; 1.6430x over previous
"""Causal GQA self-attention (B=2, S=2048, H=2048, 16 q-heads / 4 kv-heads,
head_dim=128, RoPE) as a Bass/Tile kernel on 8 TRN2 NeuronCores.

v2 sharding: hybrid batch x head-group tensor parallel. Core c owns batch
b = c//4 and head-group g = c%4 (q-heads 4g..4g+3, kv-head g -- exactly the
GQA grouping, so no kv replication). Each core computes a full [S, H]
partial of its batch's output projection; the host sums 4 partials per
batch.

On-chip layout mirrors v1 (q/k produced transposed from the PE, RoPE
rotate-half as a signed-permutation matmul, v back to natural layout via PE
transposes, scores transposed, max-free softmax) with these changes:
 - all inputs bf16 and pre-tiled on the host so every DMA is contiguous.
 - softmax denominator via DVE accumulation of the exp tiles plus a single
   ones-matmul on the accumulated sum (instead of a full ones-matmul per
   k-block): removes ~1/3 of the attention-phase PE work.
 - reciprocal_approx_fast for 1/den (~5x faster than reciprocal; den is a
   sum of exp() values in [1, ~1e4], far from the undefined edge cases).
 - output projection of the first row-half is interleaved with the second
   half's attention units to backfill the PE while ACT (exp) is busy.
 - output stored bf16, host sums partials in f32.
"""

import math

import numpy as np
import ml_dtypes

import concourse.bass as bass
import concourse.tile as tile
from concourse import mybir
from concourse.bass_utils import run_bass_kernel_spmd

F32 = mybir.dt.float32
F32R = mybir.dt.float32r
BF16 = mybir.dt.bfloat16
AF = mybir.ActivationFunctionType

B, S, H = 2, 2048, 2048
NH, NKV, HD = 16, 4, 128
N_CORES = 8
NHL = 4                # q heads per core
KT = H // 128          # 16 k-tiles over the H contraction
SC = 512               # proj s-chunk width
NSC = S // SC          # 4
QT = 1024              # attention qi tile width
NQT = S // QT          # 2
SCALE = 1.0 / math.sqrt(HD)
ACC_SPLIT = 640          # acc cols [0:640] accumulate on DVE, rest on GPSIMD
ROPE_BASE = 10000.0


def _alu(name):
    from concourse.alu_op_type import AluOpType

    return getattr(AluOpType, name)


def legalize_waits(nc, cap=1):
    """walrus in this container accepts at most one sync-wait per
    instruction; move excess waits onto NoOp carriers just before the
    instruction on the same engine (sequencers run waits in order, so this
    is semantically identical)."""
    n_split = 0
    for f in nc.m.functions:
        for blk in f.blocks:
            if not any(
                i.sync_info is not None and len(i.sync_info.on_wait) > cap
                for i in blk.instructions
            ):
                continue
            new_insts = []
            for inst in blk.instructions:
                si = inst.sync_info
                waits = list(si.on_wait) if si is not None else []
                if len(waits) > cap:
                    for k, w in enumerate(waits[:-cap]):
                        new_insts.append(
                            mybir.InstNoOp(
                                name=f"{inst.name}-wsplit{k}",
                                engine=inst.engine,
                                sync_info=mybir.SyncInfo(on_wait=[w], on_update=[]),
                            )
                        )
                        n_split += 1
                    inst.sync_info = mybir.SyncInfo(
                        on_wait=waits[-cap:], on_update=list(si.on_update)
                    )
                new_insts.append(inst)
            blk.instructions = new_insts
    return n_split


def build_nc(legalize=True):
    mult = _alu("mult")
    add = _alu("add")

    nc = bass.Bass(trn_type="TRN2", target_bir_lowering=False)

    x4_d = nc.dram_tensor("x4", [NSC, 128, KT, SC], BF16, kind="ExternalInput")
    wq_d = nc.dram_tensor("wq", [128, KT, NHL * HD], BF16, kind="ExternalInput")
    wk_d = nc.dram_tensor("wk", [128, KT, HD], BF16, kind="ExternalInput")
    wv_d = nc.dram_tensor("wv", [128, KT, HD], BF16, kind="ExternalInput")
    wo_d = nc.dram_tensor("wo", [128, NHL, H], BF16, kind="ExternalInput")
    cos_d = nc.dram_tensor("cosT", [HD, S], BF16, kind="ExternalInput")
    sinrot_d = nc.dram_tensor("sinrotT", [HD, S], F32, kind="ExternalInput")
    mask_d = nc.dram_tensor("addmask", [128, 128], BF16, kind="ExternalInput")
    rotm_d = nc.dram_tensor("rotmT", [128, 128], BF16, kind="ExternalInput")
    iden_d = nc.dram_tensor("iden", [128, 128], BF16, kind="ExternalInput")
    magic_d = nc.dram_tensor("magic", [128, QT], mybir.dt.int32, kind="ExternalInput")
    ones_d = nc.dram_tensor("ones", [128, 128], F32R, kind="ExternalInput")
    o_d = nc.dram_tensor("o", [S // 128, 128, 4, 512], BF16, kind="ExternalOutput")

    with tile.TileContext(nc) as tc:
        with (
            tc.tile_pool(name="consts", bufs=1) as consts,
            tc.tile_pool(name="xpool", bufs=2) as xpool,
            tc.tile_pool(name="homes", bufs=1) as homes,
            tc.tile_pool(name="stage", bufs=6) as stage,
            tc.tile_pool(name="tmpp", bufs=3) as tmpp,
            tc.tile_pool(name="accp", bufs=2) as accp,
            tc.tile_pool(name="denrp", bufs=2) as denrp,
            tc.tile_pool(name="ptp", bufs=4) as ptp,
            tc.tile_pool(name="opool", bufs=3) as opool,
            tc.tile_pool(name="upool", bufs=2) as upool,
            tc.tile_pool(name="ps", bufs=3, space="PSUM") as ps,
        ):
            # ---- constants (emission order = rough DMA priority) ----
            iden_sb = consts.tile([128, 128], BF16, tag="iden")
            nc.sync.dma_start(out=iden_sb, in_=iden_d.ap())
            rotm_sb = consts.tile([128, 128], BF16, tag="rotm")
            nc.sync.dma_start(out=rotm_sb, in_=rotm_d.ap())
            mask_sb = consts.tile([128, 128], BF16, tag="mask")
            nc.sync.dma_start(out=mask_sb, in_=mask_d.ap())
            wk_sb = consts.tile([128, KT, HD], BF16, tag="wk")
            nc.sync.dma_start(out=wk_sb, in_=wk_d.ap())
            wv_sb = consts.tile([128, KT, HD], BF16, tag="wv")
            nc.sync.dma_start(out=wv_sb, in_=wv_d.ap())
            xc0 = xpool.tile([128, KT, SC], BF16, tag="xc", name="xc0")
            nc.sync.dma_start(out=xc0[:, 0:KT // 2, :], in_=x4_d.ap()[0][:, 0:KT // 2, :])
            nc.sync.dma_start(out=xc0[:, KT // 2:KT, :], in_=x4_d.ap()[0][:, KT // 2:KT, :])
            wq_sb = consts.tile([128, KT, NHL * HD], BF16, tag="wq")
            nc.sync.dma_start(out=wq_sb[:, 0:KT // 2, :], in_=wq_d.ap()[:, 0:KT // 2, :])
            nc.sync.dma_start(out=wq_sb[:, KT // 2:KT, :], in_=wq_d.ap()[:, KT // 2:KT, :])
            cos_sb = consts.tile([HD, S], BF16, tag="cos")
            sinrot_sb = consts.tile([HD, S], F32, tag="sinrot")
            wo_sb = consts.tile([128, NHL, H], BF16, tag="wo")
            ones_sb = consts.tile([128, 128], F32R, tag="ones")
            magic_sb = consts.tile([128, QT], mybir.dt.int32, tag="magic")

            def load_late_consts():
                nc.sync.dma_start(out=cos_sb, in_=cos_d.ap())
                nc.sync.dma_start(out=sinrot_sb, in_=sinrot_d.ap())
                nc.sync.dma_start(out=wo_sb, in_=wo_d.ap())
                nc.sync.dma_start(out=ones_sb, in_=ones_d.ap())
                nc.sync.dma_start(out=magic_sb, in_=magic_d.ap())

            # ---- homes ----
            q_homes = [
                homes.tile([HD, S], BF16, tag=f"q{h}", name=f"q{h}_sb")
                for h in range(NHL)
            ]
            kT_sb = homes.tile([HD, S], BF16, tag="kT")
            vp_sb = homes.tile([128, KT, HD], BF16, tag="vp")
            aT_sb = homes.tile([128, NHL, S], BF16, tag="aT")

            # ---- PE warmup: keep HAM busy while real inputs stream in ----
            warm = ps.tile([128, 2 * SC], F32, tag="bank2")
            for i in range(24):
                nc.tensor.matmul(
                    warm[:, 0:128], iden_sb, iden_sb, start=True, stop=True
                )

            # ================= phase A: QKV projections + RoPE ========
            # Per chunk, PSUM "bank2" allocs rotate through 3 slots; rope
            # rotate-matmuls are deferred so the PE never waits on the ACT
            # copies that feed them.
            for c in range(NSC):
                cs = slice(c * SC, (c + 1) * SC)
                if c == 0:
                    xc = xc0
                    load_late_consts()
                else:
                    xc = xpool.tile([128, KT, SC], BF16, tag="xc")
                    nc.sync.dma_start(out=xc[:, 0:KT // 2, :], in_=x4_d.ap()[c][:, 0:KT // 2, :])
                    nc.sync.dma_start(out=xc[:, KT // 2:KT, :], in_=x4_d.ap()[c][:, KT // 2:KT, :])

                # k/v projections
                pskv = ps.tile([128, 2 * SC], F32, tag="bank2")
                for k in range(KT):
                    st = dict(start=(k == 0), stop=(k == KT - 1))
                    nc.tensor.matmul(pskv[:, 0:SC], wk_sb[:, k, :], xc[:, k, :], **st)
                    nc.tensor.matmul(pskv[:, SC:2 * SC], wv_sb[:, k, :], xc[:, k, :], **st)
                kraw = stage.tile([128, SC], BF16, tag="raw")
                nc.scalar.copy(kraw, pskv[:, 0:SC])
                vt = stage.tile([128, SC], BF16, tag="raw")
                nc.scalar.copy(vt, pskv[:, SC:2 * SC])

                # q projections head pair 0/1
                psq01 = ps.tile([128, 2 * SC], F32, tag="bank2")
                for k in range(KT):
                    st = dict(start=(k == 0), stop=(k == KT - 1))
                    nc.tensor.matmul(psq01[:, 0:SC], wq_sb[:, k, 0:HD], xc[:, k, :], **st)
                    nc.tensor.matmul(psq01[:, SC:2 * SC], wq_sb[:, k, HD:2 * HD], xc[:, k, :], **st)
                qraw0 = stage.tile([128, SC], BF16, tag="raw")
                nc.scalar.copy(qraw0, psq01[:, 0:SC])
                qraw1 = stage.tile([128, SC], BF16, tag="raw")
                nc.scalar.copy(qraw1, psq01[:, SC:2 * SC])

                # v: transpose to natural layout via PE (4x 128x128)
                pvt = ps.tile([128, 4 * SC], BF16, tag="bank2")
                for j2 in range(SC // 128):
                    nc.tensor.transpose(
                        pvt[:, j2 * 128:(j2 + 1) * 128],
                        vt[:, j2 * 128:(j2 + 1) * 128],
                        iden_sb,
                    )
                nc.vector.tensor_copy(
                    vp_sb[:, c * (SC // 128):(c + 1) * (SC // 128), :],
                    pvt[:, 0:SC],
                )
                # k rope
                psrk = ps.tile([128, 2 * SC], F32, tag="bank2")
                nc.tensor.matmul(psrk[:, 0:SC], rotm_sb, kraw, start=True, stop=True)
                ktmp = tmpp.tile([128, SC], BF16, tag="tmp")
                nc.vector.tensor_tensor(ktmp, psrk[:, 0:SC], sinrot_sb[:, cs], mult)
                nc.vector.tensor_tensor(kT_sb[:, cs], kraw, cos_sb[:, cs], mult)
                nc.vector.tensor_tensor(kT_sb[:, cs], kT_sb[:, cs], ktmp, add)

                # q projections head pair 2/3
                psq23 = ps.tile([128, 2 * SC], F32, tag="bank2")
                for k in range(KT):
                    st = dict(start=(k == 0), stop=(k == KT - 1))
                    nc.tensor.matmul(psq23[:, 0:SC], wq_sb[:, k, 2 * HD:3 * HD], xc[:, k, :], **st)
                    nc.tensor.matmul(psq23[:, SC:2 * SC], wq_sb[:, k, 3 * HD:4 * HD], xc[:, k, :], **st)
                qraw2 = stage.tile([128, SC], BF16, tag="raw")
                nc.scalar.copy(qraw2, psq23[:, 0:SC])
                qraw3 = stage.tile([128, SC], BF16, tag="raw")
                nc.scalar.copy(qraw3, psq23[:, SC:2 * SC])

                # deferred q rope rotates + combines
                for pr, (qra, qrb) in enumerate(((qraw0, qraw1), (qraw2, qraw3))):
                    psrot = ps.tile([128, 2 * SC], F32, tag="bank2")
                    nc.tensor.matmul(psrot[:, 0:SC], rotm_sb, qra, start=True, stop=True)
                    nc.tensor.matmul(psrot[:, SC:2 * SC], rotm_sb, qrb, start=True, stop=True)
                    for i, (qraw, h) in enumerate(((qra, 2 * pr), (qrb, 2 * pr + 1))):
                        qtmp = tmpp.tile([128, SC], BF16, tag="tmp")
                        nc.vector.tensor_tensor(
                            qtmp, psrot[:, i * SC:(i + 1) * SC], sinrot_sb[:, cs], mult
                        )
                        nc.vector.tensor_tensor(q_homes[h][:, cs], qraw, cos_sb[:, cs], mult)
                        nc.vector.tensor_tensor(
                            q_homes[h][:, cs], q_homes[h][:, cs], qtmp, add
                        )

            # ================= phase B/C helpers ====================
            sub = _alu("subtract")

            def attn_unit(h, t):
                """Software-pipelined: the PV matmuls of block j are emitted
                after the score matmuls of block j+1, so the PE streams
                scores while ACT runs exp(j). The causal mask is a PE
                accumulate-matmul (iden @ mask), keeping DVE/ACT off the
                score->exp critical path. exp-sums accumulate on GPSIMD."""
                qi0 = t * QT
                nblk = (qi0 + QT) // 128
                qh = q_homes[h]
                seg_touchers = {}
                for s0 in range(0, QT, SC):
                    js = [
                        j for j in range(nblk)
                        if max(j * 128 - qi0, 0) < s0 + SC
                    ]
                    seg_touchers[s0] = (js[0], js[-1])
                acc = accp.tile([128, QT], F32R, tag="acc")
                outT = ps.tile([128, QT], F32, tag="outT", bufs=1)
                pts = {}

                def emit_pv(j):
                    c0 = max(j * 128 - qi0, 0)
                    for s0 in range(0, QT, SC):
                        a0, a1 = max(c0, s0), s0 + SC
                        if a0 >= a1:
                            continue
                        jf, jl = seg_touchers[s0]
                        nc.tensor.matmul(
                            outT[:, a0:a1], vp_sb[:, j, :], pts[j][:, a0:a1],
                            start=(j == jf), stop=(j == jl),
                        )

                for j in range(nblk):
                    kj0 = j * 128
                    r = kj0 - qi0
                    c0 = max(r, 0)
                    sT = ps.tile([128, QT], F32, tag="bank2")
                    for s0 in range(0, QT, SC):
                        a0, a1 = max(c0, s0), s0 + SC
                        if a0 >= a1:
                            continue
                        diag_here = r >= 0 and a0 == c0
                        nc.tensor.matmul(
                            sT[:, a0:a1],
                            kT_sb[:, kj0:kj0 + 128],
                            qh[:, qi0 + a0:qi0 + a1],
                            start=True, stop=not diag_here,
                        )
                        if diag_here:
                            nc.tensor.matmul(
                                sT[:, c0:c0 + 128], iden_sb, mask_sb,
                                start=False, stop=True,
                            )
                    pt = ptp.tile([128, QT], BF16, tag="pt")
                    pts[j] = pt
                    nc.scalar.activation(
                        out=pt[:, c0:QT], in_=sT[:, c0:QT], func=AF.Exp, scale=SCALE
                    )
                    # exp-sum accumulation, column-split between DVE and GP
                    # (two independent serial chains on one acc tile)
                    if c0 < ACC_SPLIT:
                        if j == 0:
                            nc.vector.tensor_copy(
                                acc[:, 0:ACC_SPLIT], pt[:, 0:ACC_SPLIT]
                            )
                        else:
                            nc.vector.tensor_tensor(
                                acc[:, c0:ACC_SPLIT], acc[:, c0:ACC_SPLIT],
                                pt[:, c0:ACC_SPLIT], add,
                            )
                    g0 = max(c0, ACC_SPLIT)
                    if j == 0:
                        nc.gpsimd.tensor_copy(
                            acc[:, ACC_SPLIT:QT], pt[:, ACC_SPLIT:QT]
                        )
                    elif g0 < QT:
                        nc.gpsimd.tensor_tensor(
                            acc[:, g0:QT], acc[:, g0:QT], pt[:, g0:QT], add
                        )
                    if j > 0:
                        emit_pv(j - 1)
                        del pts[j - 1]
                emit_pv(nblk - 1)

                # evacuate outT unnormalized (ACT) so its PSUM bank frees
                # immediately; normalize lazily on DVE once den is ready.
                u_sb = upool.tile([128, QT], BF16, tag="u")
                nc.scalar.copy(u_sb, outT)

                # normalize: den = colsum(acc) via ones-matmul, then 1/den by
                # magic-constant seed (0x7EF127EA - bits, ~5% err) + one
                # Newton-Raphson pass (~0.3% worst case), standard DVE ops.
                den = ps.tile([128, QT], F32, tag="bank2")
                nc.tensor.matmul(
                    den[:, 0:SC], ones_sb, acc[:, 0:SC], start=True, stop=True
                )
                nc.tensor.matmul(
                    den[:, SC:QT], ones_sb, acc[:, SC:QT], start=True, stop=True
                )
                denr = denrp.tile([128, QT], F32, tag="denr")
                nc.vector.tensor_tensor(
                    denr.bitcast(mybir.dt.int32),
                    magic_sb,
                    den.bitcast(mybir.dt.int32),
                    sub,
                )
                dnt = denrp.tile([128, QT], F32, tag="dnt")
                nc.vector.scalar_tensor_tensor(dnt, den, -1.0, denr, mult, mult)
                nc.vector.scalar_tensor_tensor(denr, dnt, 2.0, denr, add, mult)
                nc.vector.tensor_tensor(
                    aT_sb[:, h, qi0:qi0 + QT], u_sb, denr, mult,
                )

            def oproj_tile(mt):
                ms = slice(mt * 128, (mt + 1) * 128)
                pso = None
                for n0 in range(4):
                    e0 = (n0 % 2) * 512
                    if n0 % 2 == 0:
                        pso = ps.tile([128, 1024], F32, tag="bank2")
                    for ci in range(NHL):
                        nc.tensor.matmul(
                            pso[:, e0:e0 + 512],
                            aT_sb[:, ci, ms],
                            wo_sb[:, ci, n0 * 512:(n0 + 1) * 512],
                            start=(ci == 0), stop=(ci == NHL - 1),
                        )
                    if n0 % 2 == 1:
                        os_sb = opool.tile([128, 1024], BF16, tag="os")
                        if n0 == 1:
                            nc.vector.tensor_copy(os_sb, pso)
                        else:
                            nc.scalar.copy(os_sb, pso)
                        nc.sync.dma_start(
                            out=o_d.ap()[mt, :, n0 - 1:n0 + 1, :],
                            in_=os_sb,
                        )

            # ================= phase B t0, then B t1 interleaved with C t0
            for h in range(NHL):
                attn_unit(h, 0)
            for h in range(NHL):
                attn_unit(h, 1)
                oproj_tile(2 * h)
                oproj_tile(2 * h + 1)
            for mt in range(8, 16):
                oproj_tile(mt)

    if legalize:
        legalize_waits(nc)
    return nc


_NC_CACHE = None


def _get_nc():
    global _NC_CACHE
    if _NC_CACHE is None:
        _NC_CACHE = build_nc()
    return _NC_CACHE


def _host_consts():
    inv = 1.0 / (ROPE_BASE ** (np.arange(0, HD, 2, dtype=np.float32) / HD))
    t = np.arange(S, dtype=np.float32)
    freqs = np.outer(t, inv)                       # [S, HD/2]
    emb = np.concatenate([freqs, freqs], axis=-1)  # [S, HD]
    cos = np.cos(emb)
    sin = np.sin(emb)
    cosT = np.ascontiguousarray(cos.T).astype(ml_dtypes.bfloat16)     # [HD, S]
    sinrotT = np.ascontiguousarray(sin.T).astype(np.float32)
    jj, ii = np.meshgrid(np.arange(128), np.arange(128), indexing="ij")
    addmask = np.where(jj <= ii, 0.0, -1e9).astype(ml_dtypes.bfloat16)
    # rot(q)[d] = -q[d+64] (d<64), q[d-64] (d>=64); rot = R @ q and the PE
    # computes lhsT.T @ rhs, so pass R.T as the stationary operand.
    R = np.zeros((128, 128), dtype=np.float32)
    for d in range(64):
        R[d, d + 64] = -1.0
        R[d + 64, d] = 1.0
    rotmT = np.ascontiguousarray(R.T).astype(ml_dtypes.bfloat16)
    iden = np.eye(128, dtype=np.float32).astype(ml_dtypes.bfloat16)
    return cosT, sinrotT, addmask, rotmT, iden


def _tile_w(w):
    """[H, D] f32 -> [128, KT, D] bf16 with w_t[p, k, d] = w[p + 128k, d]."""
    return np.ascontiguousarray(
        w.reshape(KT, 128, -1).transpose(1, 0, 2)
    ).astype(ml_dtypes.bfloat16)


def build_in_maps(x, wq, wk, wv, wo):
    cosT, sinrotT, addmask, rotmT, iden = _host_consts()
    # x4[b][c, p, k, s] = x[b, c*SC + s, p + 128k]
    x4 = [
        np.ascontiguousarray(
            x[b].T.reshape(KT, 128, NSC, SC).transpose(2, 1, 0, 3)
        ).astype(ml_dtypes.bfloat16)
        for b in range(B)
    ]
    in_maps = []
    for c in range(N_CORES):
        b, g = divmod(c, NHL)
        wo_slice = wo[g * NHL * HD:(g + 1) * NHL * HD, :]  # [512, H]
        in_maps.append({
            "x4": x4[b],
            "wq": _tile_w(wq[:, g * NHL * HD:(g + 1) * NHL * HD]),
            "wk": _tile_w(wk[:, g * HD:(g + 1) * HD]),
            "wv": _tile_w(wv[:, g * HD:(g + 1) * HD]),
            "wo": np.ascontiguousarray(
                wo_slice.reshape(NHL, 128, H).transpose(1, 0, 2)
            ).astype(ml_dtypes.bfloat16),
            "cosT": cosT,
            "sinrotT": sinrotT,
            "addmask": addmask,
            "rotmT": rotmT,
            "iden": iden,
            "magic": np.full((128, QT), 0x7EF127EA, dtype=np.int32),
            "ones": np.ones((128, 128), dtype=np.float32),
        })
    return in_maps


def kernel(x, wq, wk, wv, wo):
    x = np.asarray(x, dtype=np.float32)
    wq = np.asarray(wq, dtype=np.float32)
    wk = np.asarray(wk, dtype=np.float32)
    wv = np.asarray(wv, dtype=np.float32)
    wo = np.asarray(wo, dtype=np.float32)

    in_maps = build_in_maps(x, wq, wk, wv, wo)
    nc = _get_nc()
    res = run_bass_kernel_spmd(nc, in_maps, core_ids=list(range(N_CORES)))
    globals()["_LAST_RESULT"] = res
    out = np.zeros((B, S, H), dtype=np.float32)
    for c, r in enumerate(res.results):
        b = c // NHL
        out[b] += r["o"].astype(np.float32).reshape(S, H)
    return out


if __name__ == "__main__":
    rng = np.random.default_rng(0)
    ins = {
        "x": rng.standard_normal((B, S, H), dtype=np.float32),
        "wq": rng.standard_normal((H, NH * HD), dtype=np.float32) * 0.02,
        "wk": rng.standard_normal((H, NKV * HD), dtype=np.float32) * 0.02,
        "wv": rng.standard_normal((H, NKV * HD), dtype=np.float32) * 0.02,
        "wo": rng.standard_normal((NH * HD, H), dtype=np.float32) * 0.02,
    }
    out = kernel(**ins)
    print("out", out.shape, out.dtype, float(np.abs(out).max()))


# revision 5
# speedup vs baseline: 1.9154x; 1.1658x over previous
"""Causal GQA self-attention (B=2, S=2048, H=2048, 16 q-heads / 4 kv-heads,
head_dim=128, RoPE) as a Bass/Tile kernel on 8 TRN2 NeuronCores.

v2 sharding: hybrid batch x head-group tensor parallel. Core c owns batch
b = c//4 and head-group g = c%4 (q-heads 4g..4g+3, kv-head g -- exactly the
GQA grouping, so no kv replication). Each core computes a full [S, H]
partial of its batch's output projection; the host sums 4 partials per
batch.

On-chip layout mirrors v1 (q/k produced transposed from the PE, RoPE
rotate-half as a signed-permutation matmul, v back to natural layout via PE
transposes, scores transposed, max-free softmax) with these changes:
 - all inputs bf16 and pre-tiled on the host so every DMA is contiguous.
 - softmax denominator via DVE accumulation of the exp tiles plus a single
   ones-matmul on the accumulated sum (instead of a full ones-matmul per
   k-block): removes ~1/3 of the attention-phase PE work.
 - reciprocal_approx_fast for 1/den (~5x faster than reciprocal; den is a
   sum of exp() values in [1, ~1e4], far from the undefined edge cases).
 - output projection of the first row-half is interleaved with the second
   half's attention units to backfill the PE while ACT (exp) is busy.
 - output stored bf16, host sums partials in f32.
"""

import math

import numpy as np
import ml_dtypes

import concourse.bass as bass
import concourse.tile as tile
from concourse import mybir
from concourse.bass_utils import run_bass_kernel_spmd

F32 = mybir.dt.float32
F32R = mybir.dt.float32r
BF16 = mybir.dt.bfloat16
AF = mybir.ActivationFunctionType

B, S, H = 2, 2048, 2048
NH, NKV, HD = 16, 4, 128
N_CORES = 8
NHL = 4                # q heads per core
KT = H // 128          # 16 k-tiles over the H contraction
SC = 512               # proj s-chunk width
NSC = S // SC          # 4
QT = 1024              # attention qi tile width
NQT = S // QT          # 2
SCALE = 1.0 / math.sqrt(HD)
ACC_SPLIT = 640          # acc cols [0:640] accumulate on DVE, rest on GPSIMD
ROPE_BASE = 10000.0


def _alu(name):
    from concourse.alu_op_type import AluOpType

    return getattr(AluOpType, name)


def legalize_waits(nc, cap=1):
    """walrus in this container accepts at most one sync-wait per
    instruction; move excess waits onto NoOp carriers just before the
    instruction on the same engine (sequencers run waits in order, so this
    is semantically identical)."""
    n_split = 0
    for f in nc.m.functions:
        for blk in f.blocks:
            if not any(
                i.sync_info is not None and len(i.sync_info.on_wait) > cap
                for i in blk.instructions
            ):
                continue
            new_insts = []
            for inst in blk.instructions:
                si = inst.sync_info
                waits = list(si.on_wait) if si is not None else []
                if len(waits) > cap:
                    for k, w in enumerate(waits[:-cap]):
                        new_insts.append(
                            mybir.InstNoOp(
                                name=f"{inst.name}-wsplit{k}",
                                engine=inst.engine,
                                sync_info=mybir.SyncInfo(on_wait=[w], on_update=[]),
                            )
                        )
                        n_split += 1
                    inst.sync_info = mybir.SyncInfo(
                        on_wait=waits[-cap:], on_update=list(si.on_update)
                    )
                new_insts.append(inst)
            blk.instructions = new_insts
    return n_split


def build_nc(legalize=True):
    mult = _alu("mult")
    add = _alu("add")

    nc = bass.Bass(trn_type="TRN2", target_bir_lowering=False)

    x4_d = nc.dram_tensor("x4", [NSC, 128, KT, SC], BF16, kind="ExternalInput")
    wq_d = nc.dram_tensor("wq", [128, KT, NHL * HD], BF16, kind="ExternalInput")
    wk_d = nc.dram_tensor("wk", [128, KT, HD], BF16, kind="ExternalInput")
    wv_d = nc.dram_tensor("wv", [128, KT, HD], BF16, kind="ExternalInput")
    wo_d = nc.dram_tensor("wo", [128, NHL, H], BF16, kind="ExternalInput")
    cos_d = nc.dram_tensor("cosT", [HD, S], BF16, kind="ExternalInput")
    sinrot_d = nc.dram_tensor("sinrotT", [HD, S], F32, kind="ExternalInput")
    mask_d = nc.dram_tensor("addmask", [128, 128], BF16, kind="ExternalInput")
    rotm_d = nc.dram_tensor("rotmT", [128, 128], BF16, kind="ExternalInput")
    iden_d = nc.dram_tensor("iden", [128, 128], BF16, kind="ExternalInput")
    magic_d = nc.dram_tensor("magic", [128, QT], mybir.dt.int32, kind="ExternalInput")
    ones_d = nc.dram_tensor("ones", [128, 128], BF16, kind="ExternalInput")
    o_d = nc.dram_tensor("o", [S // 128, 128, 4, 512], BF16, kind="ExternalOutput")

    with tile.TileContext(nc) as tc:
        with (
            tc.tile_pool(name="consts", bufs=1) as consts,
            tc.tile_pool(name="xpool", bufs=2) as xpool,
            tc.tile_pool(name="homes", bufs=1) as homes,
            tc.tile_pool(name="stage", bufs=6) as stage,
            tc.tile_pool(name="tmpp", bufs=3) as tmpp,
            tc.tile_pool(name="accp", bufs=2) as accp,
            tc.tile_pool(name="denrp", bufs=2) as denrp,
            tc.tile_pool(name="ptp", bufs=4) as ptp,
            tc.tile_pool(name="opool", bufs=3) as opool,
            tc.tile_pool(name="upool", bufs=2) as upool,
            tc.tile_pool(name="ps", bufs=2, space="PSUM") as ps,
        ):
            # ---- constants (emission order = rough DMA priority) ----
            iden_sb = consts.tile([128, 128], BF16, tag="iden")
            nc.sync.dma_start(out=iden_sb, in_=iden_d.ap())
            rotm_sb = consts.tile([128, 128], BF16, tag="rotm")
            nc.sync.dma_start(out=rotm_sb, in_=rotm_d.ap())
            mask_sb = consts.tile([128, 128], BF16, tag="mask")
            nc.sync.dma_start(out=mask_sb, in_=mask_d.ap())
            wk_sb = consts.tile([128, KT, HD], BF16, tag="wk")
            nc.sync.dma_start(out=wk_sb, in_=wk_d.ap())
            wv_sb = consts.tile([128, KT, HD], BF16, tag="wv")
            nc.sync.dma_start(out=wv_sb, in_=wv_d.ap())
            xc0 = xpool.tile([128, KT, SC], BF16, tag="xc", name="xc0")
            nc.sync.dma_start(out=xc0[:, 0:KT // 2, :], in_=x4_d.ap()[0][:, 0:KT // 2, :])
            nc.sync.dma_start(out=xc0[:, KT // 2:KT, :], in_=x4_d.ap()[0][:, KT // 2:KT, :])
            wq_sb = consts.tile([128, KT, NHL * HD], BF16, tag="wq")
            nc.sync.dma_start(out=wq_sb[:, 0:KT // 2, :], in_=wq_d.ap()[:, 0:KT // 2, :])
            nc.sync.dma_start(out=wq_sb[:, KT // 2:KT, :], in_=wq_d.ap()[:, KT // 2:KT, :])
            cos_sb = consts.tile([HD, S], BF16, tag="cos")
            sinrot_sb = consts.tile([HD, S], F32, tag="sinrot")
            wo_sb = consts.tile([128, NHL, H], BF16, tag="wo")
            ones_sb = consts.tile([128, 128], BF16, tag="ones")
            magic_sb = consts.tile([128, QT], mybir.dt.int32, tag="magic")

            def load_late_consts():
                nc.sync.dma_start(out=cos_sb, in_=cos_d.ap())
                nc.sync.dma_start(out=sinrot_sb, in_=sinrot_d.ap())
                nc.sync.dma_start(out=wo_sb, in_=wo_d.ap())
                nc.sync.dma_start(out=ones_sb, in_=ones_d.ap())
                nc.sync.dma_start(out=magic_sb, in_=magic_d.ap())

            # ---- homes ----
            q_homes = [
                homes.tile([HD, S], BF16, tag=f"q{h}", name=f"q{h}_sb")
                for h in range(NHL)
            ]
            kT_sb = homes.tile([HD, S], BF16, tag="kT")
            vp_sb = homes.tile([128, KT, HD], BF16, tag="vp")
            aT_sb = homes.tile([128, NHL, S], BF16, tag="aT")

            # ---- PE warmup: keep HAM busy while real inputs stream in ----
            warm = ps.tile([128, 2 * SC], F32, tag="persist")
            for i in range(24):
                nc.tensor.matmul(
                    warm[:, 0:128], iden_sb, iden_sb, start=True, stop=True
                )

            # ================= phase A: QKV projections + RoPE ========
            # Per chunk, PSUM "bank2" allocs rotate through 3 slots; rope
            # rotate-matmuls are deferred so the PE never waits on the ACT
            # copies that feed them.
            for c in range(NSC):
                cs = slice(c * SC, (c + 1) * SC)
                if c == 0:
                    xc = xc0
                    load_late_consts()
                else:
                    xc = xpool.tile([128, KT, SC], BF16, tag="xc")
                    nc.sync.dma_start(out=xc[:, 0:KT // 2, :], in_=x4_d.ap()[c][:, 0:KT // 2, :])
                    nc.sync.dma_start(out=xc[:, KT // 2:KT, :], in_=x4_d.ap()[c][:, KT // 2:KT, :])

                # k/v projections
                pskv = ps.tile([128, 2 * SC], F32, tag="bank2")
                for k in range(KT):
                    st = dict(start=(k == 0), stop=(k == KT - 1))
                    nc.tensor.matmul(pskv[:, 0:SC], wk_sb[:, k, :], xc[:, k, :], **st)
                    nc.tensor.matmul(pskv[:, SC:2 * SC], wv_sb[:, k, :], xc[:, k, :], **st)
                kraw = stage.tile([128, SC], BF16, tag="raw")
                nc.scalar.copy(kraw, pskv[:, 0:SC])
                vt = stage.tile([128, SC], BF16, tag="raw")
                nc.scalar.copy(vt, pskv[:, SC:2 * SC])

                # q projections head pair 0/1
                psq01 = ps.tile([128, 2 * SC], F32, tag="bank2")
                for k in range(KT):
                    st = dict(start=(k == 0), stop=(k == KT - 1))
                    nc.tensor.matmul(psq01[:, 0:SC], wq_sb[:, k, 0:HD], xc[:, k, :], **st)
                    nc.tensor.matmul(psq01[:, SC:2 * SC], wq_sb[:, k, HD:2 * HD], xc[:, k, :], **st)
                qraw0 = stage.tile([128, SC], BF16, tag="raw")
                nc.scalar.copy(qraw0, psq01[:, 0:SC])
                qraw1 = stage.tile([128, SC], BF16, tag="raw")
                nc.scalar.copy(qraw1, psq01[:, SC:2 * SC])

                # v: transpose to natural layout via PE (4x 128x128)
                pvt = ps.tile([128, 4 * SC], BF16, tag="persist")
                for j2 in range(SC // 128):
                    nc.tensor.transpose(
                        pvt[:, j2 * 128:(j2 + 1) * 128],
                        vt[:, j2 * 128:(j2 + 1) * 128],
                        iden_sb,
                    )
                nc.vector.tensor_copy(
                    vp_sb[:, c * (SC // 128):(c + 1) * (SC // 128), :],
                    pvt[:, 0:SC],
                )
                # k rope
                psrk = ps.tile([128, 2 * SC], F32, tag="persist")
                nc.tensor.matmul(psrk[:, 0:SC], rotm_sb, kraw, start=True, stop=True)
                ktmp = tmpp.tile([128, SC], BF16, tag="tmp")
                nc.vector.tensor_tensor(ktmp, psrk[:, 0:SC], sinrot_sb[:, cs], mult)
                nc.vector.tensor_tensor(kT_sb[:, cs], kraw, cos_sb[:, cs], mult)
                nc.vector.tensor_tensor(kT_sb[:, cs], kT_sb[:, cs], ktmp, add)

                # q projections head pair 2/3
                psq23 = ps.tile([128, 2 * SC], F32, tag="bank2")
                for k in range(KT):
                    st = dict(start=(k == 0), stop=(k == KT - 1))
                    nc.tensor.matmul(psq23[:, 0:SC], wq_sb[:, k, 2 * HD:3 * HD], xc[:, k, :], **st)
                    nc.tensor.matmul(psq23[:, SC:2 * SC], wq_sb[:, k, 3 * HD:4 * HD], xc[:, k, :], **st)
                qraw2 = stage.tile([128, SC], BF16, tag="raw")
                nc.scalar.copy(qraw2, psq23[:, 0:SC])
                qraw3 = stage.tile([128, SC], BF16, tag="raw")
                nc.scalar.copy(qraw3, psq23[:, SC:2 * SC])

                # deferred q rope rotates + combines
                for pr, (qra, qrb) in enumerate(((qraw0, qraw1), (qraw2, qraw3))):
                    psrot = ps.tile([128, 2 * SC], F32, tag="persist")
                    nc.tensor.matmul(psrot[:, 0:SC], rotm_sb, qra, start=True, stop=True)
                    nc.tensor.matmul(psrot[:, SC:2 * SC], rotm_sb, qrb, start=True, stop=True)
                    for i, (qraw, h) in enumerate(((qra, 2 * pr), (qrb, 2 * pr + 1))):
                        qtmp = tmpp.tile([128, SC], BF16, tag="tmp")
                        nc.vector.tensor_tensor(
                            qtmp, psrot[:, i * SC:(i + 1) * SC], sinrot_sb[:, cs], mult
                        )
                        nc.vector.tensor_tensor(q_homes[h][:, cs], qraw, cos_sb[:, cs], mult)
                        nc.vector.tensor_tensor(
                            q_homes[h][:, cs], q_homes[h][:, cs], qtmp, add
                        )

            # ================= phase B/C helpers ====================
            sub = _alu("subtract")

            def attn_unit(h, t):
                """Software-pipelined: the PV matmuls of block j are emitted
                after the score matmuls of block j+1, so the PE streams
                scores while ACT runs exp(j). The causal mask is a PE
                accumulate-matmul (iden @ mask), keeping DVE/ACT off the
                score->exp critical path. exp-sums accumulate on GPSIMD."""
                qi0 = t * QT
                nblk = (qi0 + QT) // 128
                qh = q_homes[h]
                seg_touchers = {}
                for s0 in range(0, QT, SC):
                    js = [
                        j for j in range(nblk)
                        if max(j * 128 - qi0, 0) < s0 + SC
                    ]
                    seg_touchers[s0] = (js[0], js[-1])
                outT = ps.tile([128, QT], F32, tag="persist")
                den = ps.tile([128, QT], F32, tag="persist")
                pts = {}

                def emit_pv(j):
                    c0 = max(j * 128 - qi0, 0)
                    for s0 in range(0, QT, SC):
                        a0, a1 = max(c0, s0), s0 + SC
                        if a0 >= a1:
                            continue
                        jf, jl = seg_touchers[s0]
                        st = dict(start=(j == jf), stop=(j == jl))
                        nc.tensor.matmul(
                            outT[:, a0:a1], vp_sb[:, j, :], pts[j][:, a0:a1], **st
                        )
                        nc.tensor.matmul(
                            den[:, a0:a1], ones_sb, pts[j][:, a0:a1], **st
                        )

                for j in range(nblk):
                    kj0 = j * 128
                    r = kj0 - qi0
                    c0 = max(r, 0)
                    sT = ps.tile([128, QT], F32, tag="bank2")
                    for s0 in range(0, QT, SC):
                        a0, a1 = max(c0, s0), s0 + SC
                        if a0 >= a1:
                            continue
                        diag_here = r >= 0 and a0 == c0
                        nc.tensor.matmul(
                            sT[:, a0:a1],
                            kT_sb[:, kj0:kj0 + 128],
                            qh[:, qi0 + a0:qi0 + a1],
                            start=True, stop=not diag_here,
                        )
                        if diag_here:
                            nc.tensor.matmul(
                                sT[:, c0:c0 + 128], iden_sb, mask_sb,
                                start=False, stop=True,
                            )
                    pt = ptp.tile([128, QT], BF16, tag="pt")
                    pts[j] = pt
                    nc.scalar.activation(
                        out=pt[:, c0:QT], in_=sT[:, c0:QT], func=AF.Exp, scale=SCALE
                    )
                    if j > 0:
                        emit_pv(j - 1)
                        del pts[j - 1]
                emit_pv(nblk - 1)

                # evacuate outT unnormalized (ACT) so its PSUM bank frees
                # immediately; normalize lazily on DVE once den is ready.
                u_sb = upool.tile([128, QT], BF16, tag="u")
                nc.scalar.copy(u_sb, outT)

                # normalize: den accumulated on the PE (ones-matmuls above),
                # then 1/den by magic-constant seed (0x7EF127EA - bits, ~5%
                # err) + one Newton-Raphson pass (~0.3%), standard DVE ops.
                denr = denrp.tile([128, QT], F32, tag="denr")
                nc.vector.tensor_tensor(
                    denr.bitcast(mybir.dt.int32),
                    magic_sb,
                    den.bitcast(mybir.dt.int32),
                    sub,
                )
                dnt = denrp.tile([128, QT], F32, tag="dnt")
                nc.vector.scalar_tensor_tensor(dnt, den, -1.0, denr, mult, mult)
                nc.vector.scalar_tensor_tensor(denr, dnt, 2.0, denr, add, mult)
                nc.vector.tensor_tensor(
                    aT_sb[:, h, qi0:qi0 + QT], u_sb, denr, mult,
                )

            def oproj_tile(mt):
                ms = slice(mt * 128, (mt + 1) * 128)
                pso = None
                for n0 in range(4):
                    e0 = (n0 % 2) * 512
                    if n0 % 2 == 0:
                        pso = ps.tile([128, 1024], F32, tag="persist")
                    for ci in range(NHL):
                        nc.tensor.matmul(
                            pso[:, e0:e0 + 512],
                            aT_sb[:, ci, ms],
                            wo_sb[:, ci, n0 * 512:(n0 + 1) * 512],
                            start=(ci == 0), stop=(ci == NHL - 1),
                        )
                    if n0 % 2 == 1:
                        os_sb = opool.tile([128, 1024], BF16, tag="os")
                        if n0 == 1:
                            nc.vector.tensor_copy(os_sb, pso)
                        else:
                            nc.scalar.copy(os_sb, pso)
                        nc.sync.dma_start(
                            out=o_d.ap()[mt, :, n0 - 1:n0 + 1, :],
                            in_=os_sb,
                        )

            # ================= phase B t0, then B t1 interleaved with C t0
            for h in range(NHL):
                attn_unit(h, 0)
            for h in range(NHL):
                attn_unit(h, 1)
                oproj_tile(2 * h)
                oproj_tile(2 * h + 1)
            for mt in range(8, 16):
                oproj_tile(mt)

    if legalize:
        legalize_waits(nc)
    return nc


_NC_CACHE = None


def _get_nc():
    global _NC_CACHE
    if _NC_CACHE is None:
        _NC_CACHE = build_nc()
    return _NC_CACHE


def _host_consts():
    inv = 1.0 / (ROPE_BASE ** (np.arange(0, HD, 2, dtype=np.float32) / HD))
    t = np.arange(S, dtype=np.float32)
    freqs = np.outer(t, inv)                       # [S, HD/2]
    emb = np.concatenate([freqs, freqs], axis=-1)  # [S, HD]
    cos = np.cos(emb)
    sin = np.sin(emb)
    cosT = np.ascontiguousarray(cos.T).astype(ml_dtypes.bfloat16)     # [HD, S]
    sinrotT = np.ascontiguousarray(sin.T).astype(np.float32)
    jj, ii = np.meshgrid(np.arange(128), np.arange(128), indexing="ij")
    addmask = np.where(jj <= ii, 0.0, -1e9).astype(ml_dtypes.bfloat16)
    # rot(q)[d] = -q[d+64] (d<64), q[d-64] (d>=64); rot = R @ q and the PE
    # computes lhsT.T @ rhs, so pass R.T as the stationary operand.
    R = np.zeros((128, 128), dtype=np.float32)
    for d in range(64):
        R[d, d + 64] = -1.0
        R[d + 64, d] = 1.0
    rotmT = np.ascontiguousarray(R.T).astype(ml_dtypes.bfloat16)
    iden = np.eye(128, dtype=np.float32).astype(ml_dtypes.bfloat16)
    return cosT, sinrotT, addmask, rotmT, iden


def _tile_w(w):
    """[H, D] f32 -> [128, KT, D] bf16 with w_t[p, k, d] = w[p + 128k, d]."""
    return np.ascontiguousarray(
        w.reshape(KT, 128, -1).transpose(1, 0, 2)
    ).astype(ml_dtypes.bfloat16)


def build_in_maps(x, wq, wk, wv, wo):
    cosT, sinrotT, addmask, rotmT, iden = _host_consts()
    # x4[b][c, p, k, s] = x[b, c*SC + s, p + 128k]
    x4 = [
        np.ascontiguousarray(
            x[b].T.reshape(KT, 128, NSC, SC).transpose(2, 1, 0, 3)
        ).astype(ml_dtypes.bfloat16)
        for b in range(B)
    ]
    in_maps = []
    for c in range(N_CORES):
        b, g = divmod(c, NHL)
        wo_slice = wo[g * NHL * HD:(g + 1) * NHL * HD, :]  # [512, H]
        in_maps.append({
            "x4": x4[b],
            "wq": _tile_w(wq[:, g * NHL * HD:(g + 1) * NHL * HD]),
            "wk": _tile_w(wk[:, g * HD:(g + 1) * HD]),
            "wv": _tile_w(wv[:, g * HD:(g + 1) * HD]),
            "wo": np.ascontiguousarray(
                wo_slice.reshape(NHL, 128, H).transpose(1, 0, 2)
            ).astype(ml_dtypes.bfloat16),
            "cosT": cosT,
            "sinrotT": sinrotT,
            "addmask": addmask,
            "rotmT": rotmT,
            "iden": iden,
            "magic": np.full((128, QT), 0x7EF127EA, dtype=np.int32),
            "ones": np.ones((128, 128), dtype=ml_dtypes.bfloat16),
        })
    return in_maps


def kernel(x, wq, wk, wv, wo):
    x = np.asarray(x, dtype=np.float32)
    wq = np.asarray(wq, dtype=np.float32)
    wk = np.asarray(wk, dtype=np.float32)
    wv = np.asarray(wv, dtype=np.float32)
    wo = np.asarray(wo, dtype=np.float32)

    in_maps = build_in_maps(x, wq, wk, wv, wo)
    nc = _get_nc()
    res = run_bass_kernel_spmd(nc, in_maps, core_ids=list(range(N_CORES)))
    globals()["_LAST_RESULT"] = res
    out = np.zeros((B, S, H), dtype=np.float32)
    for c, r in enumerate(res.results):
        b = c // NHL
        out[b] += r["o"].astype(np.float32).reshape(S, H)
    return out


if __name__ == "__main__":
    rng = np.random.default_rng(0)
    ins = {
        "x": rng.standard_normal((B, S, H), dtype=np.float32),
        "wq": rng.standard_normal((H, NH * HD), dtype=np.float32) * 0.02,
        "wk": rng.standard_normal((H, NKV * HD), dtype=np.float32) * 0.02,
        "wv": rng.standard_normal((H, NKV * HD), dtype=np.float32) * 0.02,
        "wo": rng.standard_normal((NH * HD, H), dtype=np.float32) * 0.02,
    }
    out = kernel(**ins)
    print("out", out.shape, out.dtype, float(np.abs(out).max()))
